# revision 62
# baseline (speedup 1.0000x reference)
"""Trainium2 Bass kernel for nn_KSpaceTransformerGNNEncoder (8-layer graph
transformer encoder, TransformerConv(beta=True) x8 + LN + ReLU + mean-pool).

Sharding: nodes (and their incoming edges) are partitioned across 8 NeuronCores
by destination node. Each core computes Q/K/V/skip projections for its node
shard in bf16; the K|V shard is AllGathered chip-wide each layer in four
uneven chunks fired early inside the edge loop (chunk-major kv_full layout,
rotated block order so the last chunk is tiny), and per-edge K|V rows are
fetched with SWDGE gathers double-buffered four deep.  Per-edge Q is not
gathered at all: a resident fp8 one-hot mask-transpose (m4t) broadcasts each
destination block's Q to its edges with one TensorE matmul per tile, and the
ACT engine converts the PSUM result to bf16.

The hidden dimension is stored (c,h)-interleaved (j = c*HEADS + h) so that the
per-edge softmax pipeline runs entirely on packed stride-1 APs: the head
reduction is a pure contiguous-halves add tree and the v*ex broadcast reads ex
with a [0,32],[1,8] pattern whose last dim is stride-1 — both DVE 2x-mode
eligible.  Weights are permuted on the host to match.
"""
import sys
sys.path.insert(0, "/opt/trn_rl_repo")
import numpy as np

import concourse.bacc as bacc
import concourse.bass as bass
import concourse.mybir as mybir
import concourse.tile as tile
from concourse.bass import AP
from concourse.bass_utils import run_bass_kernel_spmd

# ---- problem constants (hardcoded per spec) ----
N, E, G = 20000, 320000, 64
F_IN, H, HEADS, LAYERS, OUT = 128, 256, 8, 8, 128
HEAD_C = H // HEADS
SCALE = 1.0 / np.sqrt(HEAD_C)
C = 8                     # cores
NLR = N // C              # real local nodes per core (2500)
NB = 20                   # node blocks per core
NL = NB * 128             # padded local nodes per core (2560)
NP = C * NL               # padded global nodes (20480)
P = 128
NCHUNK = 4                # AllGather pipeline chunks per layer
# uneven chunks (in blocks): chunk 0 is tiny and is the LAST one produced
# under the rotated block processing order [2..19, 0, 1], so the
# end-of-layer exposed AllGather is as small as possible
CH_BLK = [(0, 2), (2, 8), (8, 14), (14, 20)]   # [lo, hi) block ranges
CH_LROW = [(lo * P, hi * P) for lo, hi in CH_BLK]
CH_GBASE = np.cumsum([0] + [C * (hi - lo) * P for lo, hi in CH_BLK]).tolist()
P_ORD = list(range(2, NB)) + [0, 1]            # block processing order
AG_AT = {7: 1, 13: 2, 18: 3, 19: 0}            # processed-pos -> chunk issue

F32 = mybir.dt.float32
BF16 = mybir.dt.bfloat16
I32 = mybir.dt.int32
NPBF16 = mybir.dt.np(BF16)

# (c,h)-interleave permutation: new index j reads old index PERM[j]
PERM = np.array([(j % HEADS) * HEAD_C + j // HEADS for j in range(H)], np.int64)

_cache = {}


# ------------------------------------------------------------------ host prep
def _prep(x, edge_index, batch, T_blk=None):
    """Per-core index/layout preparation. Returns per-core dict arrays + T_blk."""
    src = np.asarray(edge_index[0], np.int64)
    dst = np.asarray(edge_index[1], np.int64)
    batch = np.asarray(batch, np.int64)
    x = np.asarray(x, np.float32)

    deg = np.bincount(dst, minlength=N)

    # per-core node -> (block, pos) assignment, LPT balance by in-degree
    slot_of = np.empty(N, np.int64)       # local slot within core [0, NL)
    for c in range(C):
        lo, hi = c * NLR, (c + 1) * NLR
        nodes = np.arange(lo, hi)
        order = nodes[np.argsort(-deg[lo:hi], kind="stable")]
        bin_load = np.zeros(NB, np.int64)
        bin_cnt = np.zeros(NB, np.int64)
        slots = np.empty(NLR, np.int64)
        for i, n in enumerate(order):
            open_bins = np.nonzero(bin_cnt < P)[0]
            b = open_bins[np.argmin(bin_load[open_bins])]
            slots[i] = b * P + bin_cnt[b]
            bin_load[b] += deg[n]
            bin_cnt[b] += 1
        slot_of[order] = slots
    # padded global row in the chunked-AllGather output layout (chunk-major)
    core_of = np.arange(N) // NLR
    blk_of = slot_of // P
    chunk_of = np.searchsorted([hi for _, hi in CH_BLK], blk_of, side="right")
    ch_lo = np.array([lo for lo, _ in CH_LROW])[chunk_of]
    ch_rows = np.array([hi - lo for lo, hi in CH_LROW])[chunk_of]
    ch_gb = np.array(CH_GBASE[:-1])[chunk_of]
    pad_gid = ch_gb + core_of * ch_rows + (slot_of - ch_lo)

    # per-core edge partition, block-sorted
    dst_core = dst // NLR
    per_core = []
    max_blk_tiles = 1
    for c in range(C):
        m = dst_core == c
        s_g, d_g = src[m], dst[m]
        d_slot = slot_of[d_g]
        blk = d_slot // P
        order = np.argsort(blk, kind="stable")
        s_g, d_slot, blk = s_g[order], d_slot[order], blk[order]
        cnt = np.bincount(blk, minlength=NB)
        max_blk_tiles = max(max_blk_tiles, int(np.ceil(cnt.max() / P)))
        per_core.append((c, s_g, d_slot, blk, cnt))

    if T_blk is None:
        T_blk = int(max_blk_tiles)
    assert max_blk_tiles <= T_blk
    EPB = T_blk * P

    out = []
    for c, s_g, d_slot, blk, cnt in per_core:
        src_pad = np.zeros(NB * EPB, np.int64)
        dstl = np.full((P, NB * T_blk), 999.0, NPBF16)
        pos = 0
        for b in range(NB):
            e = int(cnt[b])
            sg_b = pad_gid[s_g[pos:pos + e]]
            ds_b = d_slot[pos:pos + e]
            # non-chunk0 sources first: the first 1024-idx gather call reads
            # from kv_full[CH0:] so it need not wait for the last (chunk-0)
            # AllGather of the layer
            is_c0 = (sg_b < CH_GBASE[1]).astype(np.int64)
            ord2 = np.argsort(is_c0, kind="stable")
            sg_b, ds_b = sg_b[ord2], ds_b[ord2]
            n_nc0 = int((is_c0 == 0).sum())
            assert n_nc0 >= min(e, EPB // 2), "chunk-0 srcs spill into call A"
            sl = slice(b * EPB, b * EPB + e)
            src_pad[sl] = sg_b
            dl = (ds_b % P).astype(np.float32)
            for i in range(e):
                t = b * T_blk + i // P
                dstl[i % P, t] = dl[i]
            pos += e
            # bias call A's indices by -CH0 (pads 0 -> negative = skipped)
            a = slice(b * EPB, b * EPB + EPB // 2)
            src_pad[a] = src_pad[a] - CH_GBASE[1]
        # SWDGE gather index layout: wrapped in 16 partitions, replicated x8
        def wrap_idx(vals):
            """vals: per-block flat [NB, EPB] -> wrapped [128, NB*EPB/16]."""
            nch = vals.shape[0]
            w = np.zeros((16, nch, EPB // 16), np.int16)
            for j in range(EPB):
                w[j % 16, :, j // 16] = vals[:, j]
            w = w.reshape(16, nch * (EPB // 16))
            return np.tile(w, (8, 1))

        src_w = wrap_idx(src_pad.reshape(NB, EPB).astype(np.int16))
        # duplicated-pair dstl for 2x-mode is_equal mask generation
        dstl2 = np.repeat(dstl, 2, axis=1)   # [P, NB*T_blk*2]
        # transposed one-hot dst mask (fp8: exact 0/1), lhsT of the
        # q-broadcast matmul: m4t[d, t, e] = (dstl[e, t] == d)
        m4t = (dstl.T[None, :, :].astype(np.float32) ==
               np.arange(P, dtype=np.float32)[:, None, None])
        m4t = m4t.astype(mybir.dt.np(mybir.dt.float8e4))

        batch_f = np.full((P, NB), 999.0, np.float32)
        xT = np.zeros((F_IN, NL), NPBF16)
        lo = c * NLR
        loc_nodes = np.arange(lo, lo + NLR)
        loc_slots = slot_of[loc_nodes]
        batch_f[loc_slots % P, loc_slots // P] = batch[loc_nodes].astype(np.float32)
        xT[:, loc_slots] = x[loc_nodes].astype(NPBF16).T
        out.append(dict(src_w=src_w, m4t=m4t, dstl2=dstl2, batch_f=batch_f,
                        xT=xT))
    return out, T_blk


# ------------------------------------------------------------------ device build
def _build(T_blk):
    EPB = T_blk * P
    IDXC = EPB // 16                             # idx cols per block
    FP8 = mybir.dt.float8e4

    nc = bacc.Bacc("TRN2", target_bir_lowering=False, debug=False,
                   enable_asserts=True, num_devices=C, num_swdge_queues=4)

    # ---- external inputs
    xT_d = nc.dram_tensor("xT", [F_IN, NL], BF16, kind="ExternalInput")
    srcw_d = nc.dram_tensor("srcw", [P, NB * IDXC], mybir.dt.int16,
                            kind="ExternalInput")
    m4t_d = nc.dram_tensor("m4t", [P, NB * T_blk, P], FP8,
                           kind="ExternalInput")
    dstl2_d = nc.dram_tensor("dstl2", [P, NB * T_blk * 2], BF16,
                             kind="ExternalInput")
    batch_d = nc.dram_tensor("batchf", [P, NB], F32, kind="ExternalInput")
    iota128_d = nc.dram_tensor("iota128", [P, P], BF16, kind="ExternalInput")
    iota64_d = nc.dram_tensor("iota64", [P, G], F32, kind="ExternalInput")
    ident_d = nc.dram_tensor("ident", [P, P], BF16, kind="ExternalInput")
    identf_d = nc.dram_tensor("identf", [P, P], F32, kind="ExternalInput")
    ones_d = nc.dram_tensor("ones", [P, 1], BF16, kind="ExternalInput")
    winit_d = nc.dram_tensor("winit", [F_IN, H], BF16, kind="ExternalInput")
    binit_d = nc.dram_tensor("binit", [P, H], F32, kind="ExternalInput")
    wq_d = nc.dram_tensor("wq", [LAYERS, H, H], BF16, kind="ExternalInput")
    wk_d = nc.dram_tensor("wk", [LAYERS, H, H], BF16, kind="ExternalInput")
    wv_d = nc.dram_tensor("wv", [LAYERS, H, H], BF16, kind="ExternalInput")
    ws_d = nc.dram_tensor("ws", [LAYERS, H, H], BF16, kind="ExternalInput")
    bq_d = nc.dram_tensor("bq", [LAYERS, P, H], BF16,
                          kind="ExternalInput")
    bkv_d = nc.dram_tensor("bkv", [LAYERS, P, 2 * H], BF16,
                           kind="ExternalInput")
    bs_d = nc.dram_tensor("bs", [LAYERS, P, H], BF16,
                          kind="ExternalInput")
    u_d = nc.dram_tensor("u", [LAYERS, P, H], BF16, kind="ExternalInput")
    w_d = nc.dram_tensor("w", [LAYERS, P, H], BF16, kind="ExternalInput")
    g_d = nc.dram_tensor("g", [LAYERS, P, H], BF16, kind="ExternalInput")
    lb_d = nc.dram_tensor("lb", [LAYERS, P, H], BF16, kind="ExternalInput")
    wfin_d = nc.dram_tensor("wfin", [H, OUT], F32, kind="ExternalInput")
    bfin_d = nc.dram_tensor("bfin", [P, OUT], F32, kind="ExternalInput")

    out_d = nc.dram_tensor("out", [G, OUT], F32, kind="ExternalOutput")

    # ---- internal dram
    kv_local = [nc.dram_tensor(f"kv_local{i}", [NL, 2 * H], BF16,
                               kind="Internal") for i in range(2)]
    kv_full = [nc.dram_tensor(f"kv_full{i}", [NP, 2 * H], BF16, kind="Internal",
                              addr_space="Shared") for i in range(2)]
    pr_in = nc.dram_tensor("pr_in", [G, H + 1], F32, kind="Internal")
    pr_out = nc.dram_tensor("pr_out", [G, H + 1], F32, kind="Internal")

    with tile.TileContext(nc) as tc:
        with (
            tc.tile_pool(name="res", bufs=1) as res,       # resident constants
            tc.tile_pool(name="hT", bufs=2) as hTp,        # transposed h, ping-pong
            tc.tile_pool(name="hN", bufs=1) as hNp,        # node-major h + x_r
            tc.tile_pool(name="wl", bufs=2) as wlp,        # per-layer weights
            tc.tile_pool(name="gath", bufs=4) as gath,     # gather buffers
            tc.tile_pool(name="fin", bufs=1) as finp,      # final pool/proj
            tc.tile_pool(name="wln", bufs=1) as wlnp,      # LN/beta params
            tc.tile_pool(name="grp", bufs=1) as grp,
            tc.tile_pool(name="qep", bufs=2) as qep,       # per-group scratch
            tc.tile_pool(name="ep", bufs=2) as ep,         # epilogue scratch
            tc.tile_pool(name="ps", bufs=1, space="PSUM") as ps,
            tc.tile_pool(name="psb", bufs=3, space="PSUM") as psb,
            tc.tile_pool(name="pst", bufs=2, space="PSUM") as pst,
            tc.tile_pool(name="psq", bufs=2, space="PSUM") as psq,
        ):
            # ---------- residents
            srcw_sb = res.tile([P, NB * IDXC], mybir.dt.int16)
            m4t_sb = res.tile([P, NB * T_blk, P], FP8)
            dstl2_sb = res.tile([P, NB * T_blk * 2], BF16)
            batch_sb = res.tile([P, NB], F32)
            iota128_sb = res.tile([P, P], BF16)
            iota64_sb = res.tile([P, G], F32)
            ident_sb = res.tile([P, P], BF16)
            identf_sb = res.tile([P, P], F32)
            ones_sb = res.tile([P, 1], BF16)
            ones1_sb = res.tile([1, P], BF16)
            nc.vector.memset(ones1_sb[:], 1.0)
            eps_sb = res.tile([P, 1], F32)
            nc.vector.memset(eps_sb[:], 1e-5)
            for t, d in [(srcw_sb, srcw_d), (dstl2_sb, dstl2_d),
                         (batch_sb, batch_d),
                         (iota128_sb, iota128_d), (iota64_sb, iota64_d),
                         (ident_sb, ident_d), (identf_sb, identf_d),
                         (ones_sb, ones_d)]:
                nc.sync.dma_start(t[:], d[:, :])
            nc.sync.dma_start(m4t_sb[:], m4t_d[:, :, :])

            # ---------- per-layer weight loads (set l live from section l-1)
            wsets = []

            def load_weights(l):
                wq_sb = wlp.tile([P, 2, H], BF16, tag="wq")
                wk_sb = wlp.tile([P, 2, H], BF16, tag="wk")
                wv_sb = wlp.tile([P, 2, H], BF16, tag="wv")
                ws_sb = wlp.tile([P, 2, H], BF16, tag="ws")
                for t, d in [(wq_sb, wq_d), (wk_sb, wk_d), (wv_sb, wv_d),
                             (ws_sb, ws_d)]:
                    nc.sync.dma_start(
                        t[:], d[l].rearrange("(a p) c -> p a c", p=P))
                bq_sb = wlp.tile([P, H], BF16, tag="bq")
                bkv_sb = wlp.tile([P, 2 * H], BF16, tag="bkv")
                bs_sb = wlp.tile([P, H], BF16, tag="bs")
                for t, d in [(bq_sb, bq_d), (bkv_sb, bkv_d), (bs_sb, bs_d)]:
                    nc.sync.dma_start(t[:], d[l])
                wsets.append(dict(wq=wq_sb, wk=wk_sb, wv=wv_sb, ws=ws_sb,
                                  bq=bq_sb, bkv=bkv_sb, bs=bs_sb))

            def load_ln(l):
                """LN/beta params, single-buffered: issued from the scalar
                queue right before the layer that uses them (their WAR wait
                must not block the sync queue's stores)."""
                ws_ = wsets[l]
                u_sb = wlnp.tile([P, H], BF16, tag="u")
                w_sb = wlnp.tile([P, H], BF16, tag="w")
                g_sb = wlnp.tile([P, H], BF16, tag="g")
                lb_sb = wlnp.tile([P, H], BF16, tag="lb")
                ws_.update(u=u_sb, w=w_sb, g=g_sb, lb=lb_sb)
                for t, d in [(u_sb, u_d), (w_sb, w_d), (g_sb, g_d),
                             (lb_sb, lb_d)]:
                    nc.scalar.dma_start(t[:], d[l])

            def proj2(hT, b, w_sb, half, bias_row=None):
                """[128 nodes, H] = hT_block @ W (+ ones x bias_row), one
                region-consistent PSUM accumulation group."""
                pp = ps.tile([P, H], F32, space="PSUM", tag="pnode")
                nc.tensor.matmul(pp[:], hT[:, 0, bass.ts(b, P)], w_sb[:, 0, :],
                                 start=True, stop=False, skip_group_check=True)
                nc.tensor.matmul(pp[:], hT[:, 1, bass.ts(b, P)], w_sb[:, 1, :],
                                 start=False, stop=bias_row is None,
                                 skip_group_check=True)
                if bias_row is not None:
                    nc.tensor.matmul(pp[:], ones1_sb[:, :], bias_row,
                                     start=False, stop=True,
                                     skip_group_check=True)
                return pp

            def kvproj(l, hT, b):
                """K|V projection of block b -> kv_local[l % 2].  The bias is
                added as a rank-1 matmul (ones row x bias row) so the
                PSUM->SBUF conversion is a plain ACT copy, not a DVE add."""
                ws_ = wsets[l]
                pk = proj2(hT, b, ws_["wk"], 0, ws_["bkv"][0:1, 0:H])
                pvv = proj2(hT, b, ws_["wv"], 1, ws_["bkv"][0:1, H:2 * H])
                kvst = ep.tile([P, 2 * H], BF16, tag="kvst")
                nc.scalar.copy(kvst[:, 0:H], pk[:])
                nc.scalar.copy(kvst[:, H:2 * H], pvv[:])
                nc.sync.dma_start(kv_local[l % 2][bass.ts(b, P), :], kvst[:])

            def ag_chunk(l, j):
                """AllGather chunk j into the chunk-major kv_full layout;
                fired early so it hides behind the edge loop."""
                lo, hi = CH_LROW[j]
                nc.gpsimd.collective_compute(
                    "AllGather", mybir.AluOpType.bypass,
                    replica_groups=[list(range(C))],
                    ins=[kv_local[l % 2][lo:hi, :]],
                    outs=[kv_full[l % 2][CH_GBASE[j]:CH_GBASE[j + 1], :]])

            def qxr(l, hT, b, x_r_sb, q_sb):
                """Q and x_r projections -> SBUF, for block b."""
                ws_ = wsets[l]
                pq = proj2(hT, b, ws_["wq"], 0, ws_["bq"][0:1, :])
                nc.vector.tensor_copy(q_sb[:, b, :], pq[:])
                px = proj2(hT, b, ws_["ws"], 1, ws_["bs"][0:1, :])
                nc.vector.tensor_copy(x_r_sb[:, b, :], px[:])

            # ---------- prologue: h0 = x @ W_init, then KV(0), AG(0), Q(0)
            winit_sb = res.tile([F_IN, H], BF16)
            binit_sb = res.tile([P, H], F32)
            nc.sync.dma_start(winit_sb[:], winit_d[:, :])
            nc.sync.dma_start(binit_sb[:], binit_d[:, :])
            load_weights(0)
            load_ln(0)

            hT_cur = hTp.tile([P, 2, NL], BF16, tag="hT")
            for b in range(NB):
                xTb = finp.tile([F_IN, P], BF16, tag="xTb")
                nc.sync.dma_start(xTb[:], xT_d[:, bass.ts(b, P)])
                p0 = ps.tile([P, H], F32, space="PSUM", tag="pnode")
                nc.tensor.matmul(p0[:], xTb[:], winit_sb[:],
                                 start=True, stop=True, skip_group_check=True)
                h0 = finp.tile([P, H], BF16, tag="h0")
                nc.vector.tensor_add(h0[:], p0[:], binit_sb[:])
                for kb in range(2):
                    tp = pst.tile([P, P], BF16, space="PSUM", tag="ptr")
                    nc.tensor.transpose(tp[:], h0[:, bass.ts(kb, P)], ident_sb[:])
                    nc.vector.tensor_copy(hT_cur[:, kb, bass.ts(b, P)], tp[:])
            for b in range(NB):
                kvproj(0, hT_cur, b)
                if b in (3, 9, 15):
                    ag_chunk(0, {3: 0, 9: 1, 15: 2}[b])
                elif b == NB - 1:
                    ag_chunk(0, 3)
            x_r_sb = hNp.tile([P, NB, H], BF16, tag="x_r")
            q_sb = hNp.tile([P, NB, H], BF16, tag="q_sb")
            for b in range(NB):
                qxr(0, hT_cur, b, x_r_sb, q_sb)

            h_nm = None
            for l in range(LAYERS):
                if l < LAYERS - 1:
                    load_weights(l + 1)
                ws_ = wsets[l]
                kvf = kv_full[l % 2]
                hT_next = hTp.tile([P, 2, NL], BF16, tag="hT")
                h_nm = hNp.tile([P, NB, H], BF16, tag="h_nm")

                for pos, b in enumerate(P_ORD):
                    bT = b * T_blk
                    pv = psb.tile([P, H + 8], F32, space="PSUM", tag="pblk")
                    mm_i = 0
                    GSZ = 8
                    for gi8 in range((T_blk + GSZ - 1) // GSZ):
                        t0 = gi8 * GSZ
                        ng = min(GSZ, T_blk - t0)
                        # ------- K|V gather for this 8-tile group (SWDGE;
                        # <=1024 idxs per call: the ring holds 128
                        # descs/engine and 2048 would wedge it)
                        kvbuf = gath.tile([P, GSZ, 2 * H], BF16, tag="kvbuf")
                        nI = ng * P
                        c0 = b * IDXC + t0 * (P // 16)
                        kvsrc = kvf[CH_GBASE[1]:, :] if gi8 == 0 else kvf[:, :]
                        nc.gpsimd.dma_gather(
                            kvbuf[:, 0:ng, :],
                            kvsrc, srcw_sb[:, c0:c0 + nI // 16],
                            nI, nI, 2 * H, queue_num=(2 * pos + gi8) % 4)
                        # q per edge via mask-transpose matmul (fp8 one-hot
                        # lhsT, bf16 q rhs), copied to bf16 SBUF by the ACT
                        # engine in 2-tile chunks
                        qe = qep.tile([P, GSZ, H], BF16, tag="qe")
                        for j in range(0, ng, 2):
                            nj = min(2, ng - j)
                            qe_ps = psq.tile([P, 2, H], F32, space="PSUM",
                                             tag="qe_ps")
                            for i in range(nj):
                                nc.tensor.matmul(
                                    qe_ps[:, i, :],
                                    m4t_sb[:, bT + t0 + j + i, :],
                                    q_sb[:, b, :],
                                    start=True, stop=True,
                                    skip_group_check=True)
                            nc.scalar.copy(qe[:, j:j + nj, :],
                                           qe_ps[:, 0:nj, :])
                        # qk product, all packed bf16 (2x mode)
                        qk = grp.tile([P, GSZ, H], BF16, tag="qk")
                        nc.vector.tensor_mul(
                            qk[:, :ng, :],
                            qe[:, :ng, :],
                            kvbuf[:, 0:ng, 0:H])
                        # (c,h)-interleaved: head reduce = contiguous-halves
                        # add tree, every level packed stride-1 (2x mode)
                        t1 = grp.tile([P, GSZ, H // 2], BF16, tag="t1")
                        nc.vector.tensor_add(
                            t1[:, :ng, :], qk[:, :ng, 0:128], qk[:, :ng, 128:256])
                        t2 = grp.tile([P, GSZ, H // 4], BF16, tag="t2")
                        nc.vector.tensor_add(
                            t2[:, :ng, :], t1[:, :ng, 0:64], t1[:, :ng, 64:128])
                        t3 = grp.tile([P, GSZ, H // 8], BF16, tag="t3")
                        nc.vector.tensor_add(
                            t3[:, :ng, :], t2[:, :ng, 0:32], t2[:, :ng, 32:64])
                        t4 = grp.tile([P, GSZ, H // 16], BF16, tag="t4")
                        nc.vector.tensor_add(
                            t4[:, :ng, :], t3[:, :ng, 0:16], t3[:, :ng, 16:32])
                        alpha = grp.tile([P, GSZ, HEADS], F32, tag="alpha")
                        nc.vector.tensor_add(
                            alpha[:, :ng, :], t4[:, :ng, 0:8], t4[:, :ng, 8:16])
                        wbuf = grp.tile([P, GSZ, H + 8], BF16, tag="wbuf")
                        # ex -> wbuf[., t, 256:264] (denominator lanes)
                        nc.scalar.activation(
                            wbuf[:, :ng, H:H + 8], alpha[:, :ng, :],
                            mybir.ActivationFunctionType.Exp, scale=SCALE)
                        # wbuf = v * ex: (c,h) layout makes the ex broadcast a
                        # [0,32],[1,8] AP (last dim packed -> 2x mode)
                        nc.vector.tensor_mul(
                            wbuf[:, :ng, 0:H].rearrange(
                                "p a (c h) -> p a c h", c=HEAD_C),
                            kvbuf[:, 0:ng, H:2 * H].rearrange(
                                "p a (c h) -> p a c h", c=HEAD_C),
                            AP(tensor=wbuf[:].tensor,
                               offset=wbuf[:].offset + H,
                               ap=[wbuf[:].ap[0], [H + 8, ng], [0, HEAD_C],
                                   [1, HEADS]]))
                        # dst one-hot mask via duplicated-pair dstl (2x mode)
                        gt2 = (bT + t0) * 2
                        m4 = grp.tile([P, GSZ, P], BF16, tag="m4")
                        nc.vector.tensor_tensor(
                            m4[:, :ng, :],
                            AP(tensor=dstl2_sb[:].tensor,
                               offset=dstl2_sb[:].offset + gt2,
                               ap=[dstl2_sb[:].ap[0], [2, ng], [0, P // 2],
                                   [1, 2]]),
                            AP(tensor=iota128_sb[:].tensor,
                               offset=iota128_sb[:].offset,
                               ap=[iota128_sb[:].ap[0], [0, ng], [1, P]]),
                            op=mybir.AluOpType.is_equal)
                        for i in range(ng):
                            nc.tensor.matmul(
                                pv[:], m4[:, i, :], wbuf[:, i, :],
                                start=(mm_i == 0), stop=(mm_i == T_blk - 1),
                                skip_group_check=True)
                            mm_i += 1

                    # ------- block epilogue
                    den = ep.tile([P, HEADS], F32, tag="den")
                    nc.vector.tensor_scalar_add(den[:], pv[:, H:H + 8], 1e-16)
                    rec = ep.tile([P, HEADS], F32, tag="rec")
                    nc.vector.reciprocal(rec[:], den[:])
                    agg = ep.tile([P, H], BF16, tag="agg")
                    nc.vector.tensor_mul(
                        agg[:].rearrange("p (c h) -> p c h", c=HEAD_C),
                        pv[:, 0:H].rearrange("p (c h) -> p c h", c=HEAD_C),
                        AP(tensor=rec[:].tensor, offset=rec[:].offset,
                           ap=[rec[:].ap[0], [0, HEAD_C], [1, HEADS]]))
                    # beta = sigmoid(agg . u + x_r . w): fused mul+reduce via
                    # scalar_tensor_tensor accum_out
                    tj1 = ep.tile([P, H], F32, tag="scrf")
                    tj2 = ep.tile([P, H], F32, tag="scrf")
                    d1 = ep.tile([P, 1], F32, tag="d1")
                    d2 = ep.tile([P, 1], F32, tag="d2")
                    dlog = ep.tile([P, 1], F32, tag="dlog")
                    nc.vector.scalar_tensor_tensor(
                        out=tj1[:], in0=agg[:], scalar=1.0, in1=ws_["u"][:],
                        op0=mybir.AluOpType.mult, op1=mybir.AluOpType.mult,
                        accum_out=d1[:])
                    nc.vector.scalar_tensor_tensor(
                        out=tj2[:], in0=x_r_sb[:, b, :], scalar=1.0,
                        in1=ws_["w"][:],
                        op0=mybir.AluOpType.mult, op1=mybir.AluOpType.mult,
                        accum_out=d2[:])
                    nc.vector.tensor_add(dlog[:], d1[:], d2[:])
                    # sigmoid(x) = 1/(1+exp(-x)) -- reuses the Exp LUT, so the
                    # ACT engine avoids a ~1.3us table reload per block
                    eneg = ep.tile([P, 1], F32, tag="eneg")
                    nc.scalar.activation(eneg[:], dlog[:],
                                         mybir.ActivationFunctionType.Exp,
                                         scale=-1.0)
                    enp1 = ep.tile([P, 1], F32, tag="enp1")
                    nc.vector.tensor_scalar_add(enp1[:], eneg[:], 1.0)
                    beta = ep.tile([P, 1], F32, tag="beta")
                    nc.vector.reciprocal(beta[:], enp1[:])
                    # h = agg + beta * (x_r - agg); accum_out gives sum(h)
                    dxa = ep.tile([P, H], BF16, tag="scr")
                    nc.vector.tensor_sub(dxa[:], x_r_sb[:, b, :], agg[:])
                    hraw = ep.tile([P, H], BF16, tag="hraw")
                    s1 = ep.tile([P, 1], F32, tag="s1")
                    nc.vector.scalar_tensor_tensor(
                        out=hraw[:], in0=dxa[:], scalar=beta[:, :1], in1=agg[:],
                        op0=mybir.AluOpType.mult, op1=mybir.AluOpType.add)
                    nc.vector.reduce_sum(s1[:], hraw[:],
                                         axis=mybir.AxisListType.X)
                    # layernorm: mean + raw 2nd moment, var = E[h^2] - mu^2
                    mu = ep.tile([P, 1], F32, tag="mu")
                    nc.vector.tensor_scalar_mul(mu[:], s1[:], 1.0 / H)
                    sq = ep.tile([P, H], F32, tag="scrf")
                    s2 = ep.tile([P, 1], F32, tag="s2")
                    nc.vector.scalar_tensor_tensor(
                        out=sq[:], in0=hraw[:], scalar=1.0, in1=hraw[:],
                        op0=mybir.AluOpType.mult, op1=mybir.AluOpType.mult,
                        accum_out=s2[:])
                    musq = ep.tile([P, 1], F32, tag="musq")
                    nc.vector.tensor_mul(musq[:], mu[:], mu[:])
                    var = ep.tile([P, 1], F32, tag="var")
                    nc.vector.scalar_tensor_tensor(
                        out=var[:], in0=s2[:], scalar=1.0 / H, in1=musq[:],
                        op0=mybir.AluOpType.mult,
                        op1=mybir.AluOpType.subtract)
                    sd = ep.tile([P, 1], F32, tag="sd")
                    nc.scalar.activation(sd[:], var[:],
                                         mybir.ActivationFunctionType.Sqrt,
                                         bias=eps_sb[:, :1])
                    rstd = ep.tile([P, 1], F32, tag="rstd")
                    nc.vector.reciprocal(rstd[:], sd[:])
                    hgv = ep.tile([P, H], BF16, tag="scr")
                    nc.vector.scalar_tensor_tensor(
                        out=hgv[:], in0=hraw[:], scalar=mu[:, :1],
                        in1=ws_["g"][:],
                        op0=mybir.AluOpType.subtract, op1=mybir.AluOpType.mult)
                    hb2 = ep.tile([P, H], BF16, tag="hb2")
                    nc.vector.scalar_tensor_tensor(
                        out=hb2[:], in0=hgv[:], scalar=rstd[:, :1],
                        in1=ws_["lb"][:],
                        op0=mybir.AluOpType.mult, op1=mybir.AluOpType.add)
                    nc.vector.tensor_scalar_max(h_nm[:, b, :], hb2[:], 0.0)
                    for kb in range(2):
                        tp = pst.tile([P, P], BF16, space="PSUM", tag="ptr")
                        nc.tensor.transpose(tp[:], h_nm[:, b, bass.ts(kb, P)],
                                            ident_sb[:])
                        nc.vector.tensor_copy(hT_next[:, kb, bass.ts(b, P)],
                                              tp[:])
                    # interleave next layer's K|V projection; AllGather
                    # chunks are issued a couple of blocks after their data
                    # is ready so the Pool queue's wait doesn't stall gather
                    # desc-gen
                    if l < LAYERS - 1:
                        kvproj(l + 1, hT_next, b)
                        if pos in AG_AT:
                            ag_chunk(l + 1, AG_AT[pos])

                if l < LAYERS - 1:
                    load_ln(l + 1)
                    x_r_sb = hNp.tile([P, NB, H], BF16, tag="x_r")
                    q_sb = hNp.tile([P, NB, H], BF16, tag="q_sb")
                    for b in range(NB):
                        qxr(l + 1, hT_next, b, x_r_sb, q_sb)
                hT_cur = hT_next

            # ---------- global mean pool + final proj
            pp_sum = ps.tile([G, H], F32, space="PSUM", tag="pnode")
            pp_cnt = pst.tile([G, 8], F32, space="PSUM", tag="ptr")
            for b in range(NB):
                bmat = finp.tile([P, G], BF16, tag="bmat")
                nc.vector.tensor_scalar(bmat[:], iota64_sb[:],
                                        batch_sb[:, b:b + 1], None,
                                        mybir.AluOpType.is_equal)
                nc.tensor.matmul(pp_sum[:], bmat[:], h_nm[:, b, :],
                                 start=(b == 0), stop=(b == NB - 1),
                                 skip_group_check=True)
                nc.tensor.matmul(pp_cnt[:, 0:1], bmat[:], ones_sb[:],
                                 start=(b == 0), stop=(b == NB - 1),
                                 skip_group_check=True)
            pool_sb = finp.tile([G, H + 1], F32, tag="pool_sb")
            nc.vector.tensor_copy(pool_sb[:, 0:H], pp_sum[:])
            nc.vector.tensor_copy(pool_sb[:, H:H + 1], pp_cnt[:, 0:1])
            nc.gpsimd.dma_start(pr_in[:, :], pool_sb[:])
            nc.gpsimd.collective_compute(
                "AllReduce", mybir.AluOpType.add,
                replica_groups=[list(range(C))],
                ins=[pr_in[:, :]], outs=[pr_out[:, :]])
            red_sb = finp.tile([G, H + 1], F32, tag="red_sb")
            nc.sync.dma_start(red_sb[:], pr_out[:, :])
            cnt = finp.tile([G, 1], F32, tag="cnt")
            nc.vector.tensor_scalar_max(cnt[:], red_sb[:, H:H + 1], 1.0)
            cinv = finp.tile([G, 1], F32, tag="cinv")
            nc.vector.reciprocal(cinv[:], cnt[:])
            pooled = finp.tile([G, H], F32, tag="pooled")
            nc.vector.tensor_scalar_mul(pooled[:], red_sb[:, 0:H], cinv[:, :1])
            poolT = finp.tile([P, 2, G], F32, tag="poolT")
            for kb in range(2):
                tp = pst.tile([P, G], F32, space="PSUM", tag="ptr")
                nc.tensor.transpose(tp[:], pooled[:, bass.ts(kb, P)],
                                    identf_sb[0:G, 0:G])
                nc.vector.tensor_copy(poolT[:, kb, :], tp[:])
            wfin_sb = finp.tile([P, 2, OUT], F32, tag="wfin_sb")
            nc.sync.dma_start(wfin_sb[:],
                              wfin_d[:, :].rearrange("(a p) c -> p a c", p=P))
            bfin_sb = finp.tile([P, OUT], F32, tag="bfin_sb")
            nc.sync.dma_start(bfin_sb[:], bfin_d[:, :])
            pf = ps.tile([G, OUT], F32, space="PSUM", tag="pnode")
            nc.tensor.matmul(pf[:], poolT[:, 0, :], wfin_sb[:, 0, :],
                             start=True, stop=False, skip_group_check=True)
            nc.tensor.matmul(pf[:], poolT[:, 1, :], wfin_sb[:, 1, :],
                             start=False, stop=True, skip_group_check=True)
            fin = finp.tile([G, OUT], F32, tag="fin")
            nc.vector.tensor_add(fin[:], pf[:], bfin_sb[0:G, :])
            nc.sync.dma_start(out_d[:, :], fin[:])

    nc.compile()
    return nc


# ------------------------------------------------------------------ entry point
def kernel(x, edge_index, batch, W_init, b_init, Wq, bq, Wk, bk, Wv, bv,
           Ws, bs, Wbeta, ln_g, ln_b, W_final, b_final, _trace=False):
    per_core, T_blk = _prep(x, edge_index, batch)
    if T_blk not in _cache:
        _cache[T_blk] = _build(T_blk)
    nc = _cache[T_blk]

    rep = lambda v: np.tile(np.asarray(v, np.float32)[None, :], (P, 1))
    bf = lambda v: np.asarray(v, np.float32).astype(NPBF16)
    Wbeta = np.asarray(Wbeta, np.float32)
    u = Wbeta[:, 0:H, 0] + Wbeta[:, 2 * H:3 * H, 0]
    w = Wbeta[:, H:2 * H, 0] - Wbeta[:, 2 * H:3 * H, 0]
    bkv = np.concatenate([np.asarray(bk, np.float32),
                          np.asarray(bv, np.float32)], axis=1)
    # (c,h)-interleave: rows of every [H,*] matrix and entries of every [H]
    # vector permuted; columns of every [*,H] matrix permuted.  LayerNorm and
    # the beta dot products are permutation-invariant.
    pc_ = lambda W: np.asarray(W, np.float32)[..., PERM]         # cols
    prc = lambda W: np.asarray(W, np.float32)[:, PERM, :][..., PERM]
    pv_ = lambda v: np.asarray(v, np.float32)[..., PERM]
    bkv_p = np.concatenate([pv_(np.asarray(bk, np.float32)),
                            pv_(np.asarray(bv, np.float32))], axis=1)
    shared = dict(
        iota128=np.tile(np.arange(P, dtype=np.float32)[None, :],
                        (P, 1)).astype(NPBF16),
        iota64=np.tile(np.arange(G, dtype=np.float32)[None, :], (P, 1)),
        ident=np.eye(P, dtype=np.float32).astype(NPBF16),
        identf=np.eye(P, dtype=np.float32),
        ones=np.ones((P, 1), np.float32).astype(NPBF16),
        winit=bf(pc_(W_init)),
        binit=rep(pv_(b_init)),
        wq=bf(prc(Wq)), wk=bf(prc(Wk)), wv=bf(prc(Wv)), ws=bf(prc(Ws)),
        bq=np.stack([rep(pv_(bq)[l]) for l in range(LAYERS)]).astype(NPBF16),
        bkv=np.stack([rep(bkv_p[l]) for l in range(LAYERS)]).astype(NPBF16),
        bs=np.stack([rep(pv_(bs)[l]) for l in range(LAYERS)]).astype(NPBF16),
        u=np.stack([rep(pv_(u)[l]) for l in range(LAYERS)]).astype(NPBF16),
        w=np.stack([rep(pv_(w)[l]) for l in range(LAYERS)]).astype(NPBF16),
        g=np.stack([rep(pv_(ln_g)[l]) for l in range(LAYERS)]).astype(NPBF16),
        lb=np.stack([rep(pv_(ln_b)[l]) for l in range(LAYERS)]).astype(NPBF16),
        wfin=np.asarray(W_final, np.float32)[PERM, :],
        bfin=rep(b_final),
    )
    in_maps = []
    for c in range(C):
        pc = per_core[c]
        in_maps.append(dict(shared, xT=pc["xT"], srcw=pc["src_w"],
                            m4t=pc["m4t"], dstl2=pc["dstl2"],
                            batchf=pc["batch_f"]))
    res = run_bass_kernel_spmd(nc, in_maps, core_ids=list(range(C)),
                               trace=_trace)
    out = res.results[0]["out"]
    if _trace:
        kernel._last_exec_ns = res.exec_time_ns
    return out


if __name__ == "__main__":
    pass
